# revision 34
# baseline (speedup 1.0000x reference)
"""Trainium2 Bass kernel for the BERT span-pair classifier problem.

Computes, for B=2 batches over a 252x252 span-pair grid:
    h    = relu(Ai[i] + Aj[j] + ind(i,j)*w1c + b1)        # [770] per pair
    out  = h @ W2.T + b2                                   # [36]  per pair
    out  = where(span_mask >= 1, out, 0)
    res  = log_softmax(out over the 63504 pairs)           # per (batch, label)
    return res transposed to [B, 36, L*L]

v2 strategy (8 NeuronCores, SPMD single program):
  - Host precomputes Ai/Aj (O(L*770) matmuls) and ships per-core tensors:
      bj0b  : Aj + b1 per (chunk, batch, j)                       (shared)
      bjwt  : per-core in-span window rows with ai, w1c*ind baked  (static
              placement -> window overwrites are plain static relu-copies)
      aib   : per-(chunk, slot) Ai columns (tensor_scalar biases)
      hts6q : tail rows [h768, h769, m, 1-m] fully host-computed
      maskrep: mask broadcast over 36 partitions for the masked copy
      cnts  : per-core invalid-pair counts (for the softmax denominator)
  - Device work per 2-slot tile: 12 relu tensor_scalar ops (h build, spread
    over DVE/ACT/Pool via a greedy static load balancer), 6+1 bf16 matmuls
    (W2 chunks + host-baked tail with b2*m + BIGNEG*(1-m) rows), one packed
    exp+accum and one masked psum->SBUF multiply per 2-tile psum bank.
  - Two tiles share one PSUM bank (partitions 0-35 / 64-99), so exp /
    mask-mult / final(-LSE) ops run at [100, 504] granularity.
  - Each core ships raw per-group exp sums; the host combines them with the
    (host-known) invalid-pair counts, takes the log, and applies the final
    -LSE subtraction during unshard. No device collective, no device tail:
    stores stream out group by group during the main loop.
"""

import math
import os
from contextlib import ExitStack

import numpy as np

import concourse.bass as bass
import concourse.bacc as bacc
import concourse.tile as tile
from concourse import mybir
from concourse._compat import with_exitstack
from concourse.bass_utils import run_bass_kernel_spmd

L = 252
HID = 768
MLP = 770
NLAB = 36
B = 2
NC = 8
KC = 6            # full 128-row hid chunks (6*128 = 768)
W2SCALE = 16.0    # fp8 W2 is shipped pre-scaled; host divides logits by 16

FP32 = mybir.dt.float32
BF16 = mybir.dt.bfloat16
FP8 = mybir.dt.float8e4
AF = mybir.ActivationFunctionType
ALU = mybir.AluOpType

SLOTW = 252       # per-slot j width inside an h tile


def plan_slots(spans):
    """Slot layout: [in0, off0(+pad), in1, off1(+pad)]; per-batch slot count
    padded to a multiple of 4 so 2-tile psum groups are batch-pure."""
    segs = []
    slot = 0
    for b in range(B):
        s, e = spans[b]
        n = e - s + 1
        nin = math.ceil(n / NC)
        noff = math.ceil((L - n) / NC)
        pad = (-(nin + noff)) % 4
        segs.append(dict(kind="in", b=b, start=slot, nslots=nin, s=s, e=e,
                         count=n))
        slot += nin
        rows = [r for r in range(L) if r < s or r > e]
        segs.append(dict(kind="off", b=b, start=slot, nslots=noff + pad,
                         rows=rows, count=len(rows)))
        slot += noff + pad
    nslot = slot
    assert nslot % 4 == 0
    return segs, nslot


def slot_map_for_core(segs, nslot, c):
    """-> list over slots of (batch, global_row) or None for padding."""
    m = [None] * nslot
    for sg in segs:
        for k in range(sg["nslots"]):
            idx = NC * k + c
            p = sg["start"] + k
            if idx < sg["count"]:
                if sg["kind"] == "in":
                    m[p] = (sg["b"], sg["s"] + idx)
                else:
                    m[p] = (sg["b"], sg["rows"][idx])
    return m


def window_layout(segs):
    """Static (compile-time) ragged layout of the in-span window rows.

    Returns list of (slot, batch, k, j0, W, woff) and total width WTOT.
    Window for in-span slot k of batch b: columns [j0, j0+W) with
    j0 = s + 8k, W = min(e - s - 8k + 9, SLOTW - j0), covering [i_c, e]
    for every core offset c in [0, 8).
    """
    ents = []
    off = 0
    for sg in segs:
        if sg["kind"] != "in":
            continue
        s, e = sg["s"], sg["e"]
        for k in range(sg["nslots"]):
            j0 = s + NC * k
            W = min(e - s - NC * k + 9, SLOTW - j0)
            W = max(W, 1)
            ents.append(dict(slot=sg["start"] + k, b=sg["b"], k=k, j0=j0,
                             W=W, woff=off, s=s, e=e))
            off += W
    return ents, off


def build_kernel(segs, nslot, wents, WTOT, plan):
    ntile = nslot // 2
    ngrp = ntile // 2
    slot_batch = []
    for sg in segs:
        slot_batch.extend([sg["b"]] * sg["nslots"])
    grp_batch = [slot_batch[4 * g] for g in range(ngrp)]
    for g in range(ngrp):
        assert len({slot_batch[4 * g + i] for i in range(4)}) == 1
    # batch -> contiguous group range
    b0g = sum(1 for b in grp_batch if b == 0)
    assert all(b == 0 for b in grp_batch[:b0g])
    wents_by_slot = {w["slot"]: w for w in wents}

    @with_exitstack
    def kern(ctx: ExitStack, tc: tile.TileContext, outs, ins):
        nc = tc.nc
        bj0b = ins["bj0b"]        # [6*128, 1024] bf16 (chunk-major)
        bjwt = ins["bjwt"]        # [6*128, WTOT] fp8 (chunk-major)
        aib = ins["aib"]          # [6*128, nslot] bf16 (chunk-major)
        w2c6 = ins["w2c6"]        # [128, 3*2*36] fp8 (DoubleRow pairs, x16)
        outd = outs["out"]        # [ntile*36, 504] bf16 (tile-major rows)

        fp = ctx.enter_context(tc.tile_pool(name="fp", bufs=1))
        hp = ctx.enter_context(tc.tile_pool(name="hp", bufs=4))
        psp = ctx.enter_context(tc.tile_pool(name="psp", bufs=1, space="PSUM"))

        # ---- persistent SBUF ----
        s_bj0 = fp.tile([128, KC * 1024], BF16)
        s_bjw = fp.tile([128, KC * WTOT], FP8)
        s_ai = fp.tile([128, KC * nslot], FP32)
        s_aib = fp.tile([128, KC * nslot], BF16)
        s_w2 = fp.tile([128, KC * 64], FP8)
        s_out = fp.tile([NLAB, ntile * 504], BF16)

        # ---- load constants (many small pieces -> spread over DMA queues,
        # ordered so early-group data lands first) ----
        q = [nc.sync, nc.gpsimd]
        nq = 0

        def ld(dst, src):
            nonlocal nq
            q[nq % 2].dma_start(out=dst, in_=src)
            nq += 1

        ld(s_w2, w2c6)
        WP = 3

        def ldw(c, wpc):
            w0 = (WTOT // WP) * wpc
            w1 = (WTOT // WP) * (wpc + 1) if wpc < WP - 1 else WTOT
            ld(s_bjw[:, WTOT * c + w0:WTOT * c + w1],
               bjwt[128 * c:128 * (c + 1), w0:w1])

        for c in range(KC):
            ld(s_aib[:, nslot * c:nslot * (c + 1)],
               aib[128 * c:128 * (c + 1), :])
            nc.vector.tensor_copy(out=s_ai[:, nslot * c:nslot * (c + 1)],
                                  in_=s_aib[:, nslot * c:nslot * (c + 1)])
            ld(s_bj0[:, 1024 * c:1024 * (c + 1)],
               bj0b[128 * c:128 * (c + 1), :])
            ldw(c, 0)
        for m in range(WP - 1):
            for c in range(KC):
                ldw(c, m + 1)

        def ts_relu(eng, out, in0, sc):
            if eng is nc.scalar:
                nc.scalar.activation(out, in0, AF.Relu, bias=sc, scale=1.0)
            else:
                eng.tensor_scalar(out=out, in0=in0, scalar1=sc, scalar2=0.0,
                                  op0=ALU.add, op1=ALU.max)

        # ---- main loop over tiles: one psum bank per tile ----
        psb = [psp.tile([NLAB, 2 * L], FP32, tag=f"ps{i}", name=f"ps{i}")
               for i in range(8)]

        def emit_copy(tc_):
            psc = psb[tc_ % 8]
            ceng = plan(tc_, 0, 0)
            seg = s_out[:, 2 * L * tc_:2 * L * (tc_ + 1)]
            if ceng is nc.scalar:
                nc.scalar.activation(seg, psc, AF.Identity, scale=1.0)
            else:
                nc.vector.tensor_copy(out=seg, in_=psc)
            nc.gpsimd.dma_start(out=outd[NLAB * tc_:NLAB * (tc_ + 1), :],
                                in_=seg)

        for t in range(ntile):
            ps = psb[t % 8]
            hh = hp.tile([128, KC * 512], FP8, tag="hh",
                         name=f"hh{t}")
            for sl in range(2):
                p = 2 * t + sl
                b = slot_batch[p]
                went = wents_by_slot.get(p)
                for c in range(KC):
                    eng = plan(t, c, sl)
                    ho = 512 * c + SLOTW * sl
                    ts_relu(eng, hh[:, ho:ho + L],
                            s_bj0[:, 1024 * c + 512 * b:
                                  1024 * c + 512 * b + L],
                            s_ai[:, nslot * c + p:nslot * c + p + 1])
                    if went is not None:
                        # window rows are shipped pre-relu'd fp8: the
                        # overwrite is a plain copy (max(x,0) is a no-op)
                        wo = WTOT * c + went["woff"]
                        weng = plan(t, c, sl + 2)
                        wdst = hh[:, ho + went["j0"]:
                                  ho + went["j0"] + went["W"]]
                        wsrc = s_bjw[:, wo:wo + went["W"]]
                        if weng is nc.scalar:
                            nc.scalar.activation(wdst, wsrc, AF.Relu,
                                                 scale=1.0)
                        else:
                            weng.tensor_scalar(
                                out=wdst, in0=wsrc, scalar1=0.0,
                                scalar2=None, op0=ALU.max)
            # matmuls: 3 fp8 DoubleRow chunk-pairs + host-baked tail
            for qq in range(KC // 2):
                rhs = hh[:, 1024 * qq:1024 * (qq + 1)].rearrange(
                    "p (k sw) -> p k sw", k=2)[:, :, 0:2 * L]
                lhs = s_w2[:, 128 * qq:128 * (qq + 1)].rearrange(
                    "p (k n) -> p k n", k=2)[:, :, 0:NLAB]
                nc.tensor.matmul(ps, lhs, rhs,
                                 perf_mode=mybir.MatmulPerfMode.DoubleRow,
                                 start=(qq == 0), stop=(qq == KC // 2 - 1),
                                 skip_group_check=True)
            # raw partial logits -> bf16 staging with a 2-tile emission
            # delay (the psum read then never waits at an engine queue
            # head), then straight out to HBM; the 770-tail + b2, mask,
            # exp-sums and -LSE are applied host-side
            if t >= 2:
                emit_copy(t - 2)
        emit_copy(ntile - 2)
        emit_copy(ntile - 1)

    return kern, ngrp


def make_plan(nc_getter, segs, nslot, wents):
    """Greedy static load balancer for the h-build ops."""
    ntile = nslot // 2
    wents_by_slot = {w["slot"]: w for w in wents}
    # preload other duties (ns): ACT: exp+copy+accum; DVE: memsets+casts
    # Pool excluded: measured ~4.6us per tensor op (Q7 emulation).
    # Tile-granular assignment: all h ops of a tile go to ONE engine so the
    # consuming matmuls' waits are satisfied in issue order (no sequencer
    # head-of-line blocking on scattered cross-engine deps).
    load = {"v": 2000.0, "a": 1500.0}

    table = {}
    for t in range(ntile):
        wins = [wents_by_slot[2 * t + sl] for sl in range(2)
                if (2 * t + sl) in wents_by_slot]
        cv = 12 * 261.0 + 548.0 + sum(
            KC * (0.52 * w["W"] + 60.0) for w in wins)
        ca = 12 * 350.0 + 674.0 + sum(
            KC * (0.833 * w["W"] + 110.0) for w in wins)
        k = "v" if load["v"] + cv <= load["a"] + ca else "a"
        load[k] += cv if k == "v" else ca
        table[t] = k

    def plan(t, c, sl):
        nc = nc_getter()
        return {"v": nc.vector, "a": nc.scalar}[table[t]]

    return plan


def kernel(**inputs) -> np.ndarray:
    hidden = np.asarray(inputs["hidden"], dtype=np.float32)
    pred_spans = np.asarray(inputs["pred_spans"]).astype(np.int64)
    span_mask = np.asarray(inputs["span_mask"]).astype(np.int32)
    W1 = np.asarray(inputs["W1"], dtype=np.float32)
    b1 = np.asarray(inputs["b1"], dtype=np.float32)
    W2 = np.asarray(inputs["W2"], dtype=np.float32)
    b2 = np.asarray(inputs["b2"], dtype=np.float32)

    spans = [(int(pred_spans[b, 0]), int(pred_spans[b, 1])) for b in range(B)]
    segs, nslot = plan_slots(spans)
    ntile = nslot // 2
    ngrp = ntile // 2
    wents, WTOT = window_layout(segs)
    wents_by_slot = {w["slot"]: w for w in wents}

    vecs = hidden[:, 1:L + 1, :]                       # [B, L, 768]
    W1T = W1.T                                         # [1537, 770]
    w1c = np.ascontiguousarray(W1T[2 * HID])           # [770]
    # host prep: Ai/Aj for all rows/cols
    Aj = np.einsum("bld,dh->blh", vecs, W1T[HID:2 * HID])   # [B, L, 770]
    Ai = np.einsum("bld,dh->blh", vecs, W1T[0:HID])         # [B, L, 770]
    Bj0 = Aj + b1[None, None, :]                            # [B, L, 770]

    W2T = np.ascontiguousarray(W2.T)                   # [770, 36]
    maskf = span_mask.astype(np.float32).clip(0, 1)    # [252, 252]

    bf = mybir.dt.np(BF16)
    f8 = mybir.dt.np(FP8)

    # shared tensors
    bj0b = np.zeros((128, KC, 2, 512), np.float32)
    for c in range(KC):
        for b in range(B):
            bj0b[:, c, b, 0:L] = Bj0[b, :, 128 * c:128 * (c + 1)].T
    bj0b = np.ascontiguousarray(
        bj0b.transpose(1, 0, 2, 3)).reshape(KC * 128, 1024)

    # fp8 DoubleRow stationary: (p, pair q, k, n) = W2T[256q + 128k + p, n],
    # k-stride padded to 64 cols for the 16B ldweights alignment rule
    w2c6 = np.zeros((128, KC // 2, 2, 64), np.float32)
    for qq in range(KC // 2):
        for k in range(2):
            r0 = 256 * qq + 128 * k
            w2c6[:, qq, k, 0:NLAB] = W2T[r0:r0 + 128] * W2SCALE
    w2c6 = w2c6.reshape(128, KC * 64)

    in_maps = []
    slot_maps = []
    core_cnts = []
    for core in range(NC):
        sm = slot_map_for_core(segs, nslot, core)
        slot_maps.append(sm)

        # aib: per-(chunk, slot) Ai columns
        aib = np.zeros((128, KC, nslot), np.float32)
        for p, ent in enumerate(sm):
            if ent is None:
                continue
            b, r = ent
            for c in range(KC):
                aib[:, c, p] = Ai[b, r, 128 * c:128 * (c + 1)]

        # bjwt: in-span window rows, everything baked (ai + w1c*ind [+E2])
        bjwt = np.zeros((128, KC, WTOT), np.float32)
        for w in wents:
            b = w["b"]
            s, e, k = w["s"], w["e"], w["k"]
            i = s + NC * k + core
            ent = sm[w["slot"]]
            js = np.arange(w["j0"], w["j0"] + w["W"])
            jc = np.clip(js, 0, L - 1)
            ind = ((js >= i) & (js <= e)).astype(np.float32)
            if k == 0 and core == 0:
                ind[js == e] = 2.0
            valid = (js < L).astype(np.float32)
            if ent is None:
                ai_row = np.zeros((MLP,), np.float32)
                ind = ind * 0.0
            else:
                ai_row = Ai[b, i]
            for c in range(KC):
                rows = slice(128 * c, 128 * (c + 1))
                vals = (Bj0[b, jc, 128 * c:128 * (c + 1)].T
                        + ai_row[rows, None]
                        + w1c[rows, None] * ind[None, :]) * valid[None, :]
                bjwt[:, c, w["woff"]:w["woff"] + w["W"]] = np.maximum(vals, 0)
        bjwt = np.ascontiguousarray(bjwt.transpose(1, 0, 2)).reshape(KC * 128, WTOT)

        # invalid-pair counts for this core's real rows, per batch
        cnt = np.zeros((2,), np.float64)
        for p, ent in enumerate(sm):
            if ent is None:
                continue
            b, r = ent
            cnt[b] += L - maskf[r].sum()

        core_cnts.append(cnt)
        in_maps.append({
            "bj0b": bj0b.astype(bf), "bjwt": bjwt.astype(f8),
            "aib": np.ascontiguousarray(
                aib.transpose(1, 0, 2)).reshape(KC * 128, nslot).astype(bf),
            "w2c6": w2c6.astype(f8),
        })

    # ---- build program ----
    nc = bacc.Bacc("TRN2", target_bir_lowering=False, debug=False,
                   enable_asserts=False, num_devices=NC)

    def mk(name, arr, dt):
        return nc.dram_tensor(name, list(arr.shape), dt,
                              kind="ExternalInput").ap()

    ex = in_maps[0]
    ins_aps = {
        "bj0b": mk("bj0b", ex["bj0b"], BF16),
        "bjwt": mk("bjwt", ex["bjwt"], FP8),
        "aib": mk("aib", ex["aib"], BF16),
        "w2c6": mk("w2c6", ex["w2c6"], FP8),
    }
    outs_aps = {
        "out": nc.dram_tensor("out", [(nslot // 2) * NLAB, 504], BF16,
                              kind="ExternalOutput").ap(),
    }

    plan = make_plan(lambda: nc, segs, nslot, wents)
    kern, ngrp_chk = build_kernel(segs, nslot, wents, WTOT, plan)
    with tile.TileContext(nc) as t:
        kern(t, outs_aps, ins_aps)
    nc.compile()

    if os.environ.get("BK_BUILD_ONLY"):
        print("BUILD OK")
        return np.zeros((B, NLAB, L * L), np.float32)

    if os.environ.get("BK_SIM"):
        from concourse.bass_interp import MultiCoreSim

        sim = MultiCoreSim(nc, num_cores=NC, require_finite=False,
                           require_nnan=False)
        for c, cs in sim.cores.items():
            for name, arr in in_maps[c].items():
                cs.tensor(name)[:] = arr
            if nc.partition_id_tensor is not None:
                cs.tensor(nc.partition_id_tensor.name)[:] = np.array(
                    [[c]], dtype=np.uint32)
        sim.simulate(check_with_hw=False)

        class _R:
            results = [{"out": np.asarray(sim.cores[c].tensor("out"))}
                       for c in range(NC)]
        res = _R()
    else:
        trace = bool(int(os.environ.get("BK_TRACE", "0")))
        res = run_bass_kernel_spmd(nc, in_maps, core_ids=list(range(NC)),
                                   trace=trace)
        if trace and res.exec_time_ns is not None:
            print(f"HW exec time: {res.exec_time_ns} ns")

    # ---- unshard + host-side mask / log-sum-exp / -LSE ----
    ngrp = nslot // 4
    slot_batch = []
    for sg in segs:
        slot_batch.extend([sg["b"]] * sg["nslots"])
    grp_batch = [slot_batch[4 * g] for g in range(ngrp)]

    raw = np.zeros((B, NLAB, L * L), np.float32)
    ntile = nslot // 2
    for core in range(NC):
        oc = res.results[core]["out"].astype(np.float32) / W2SCALE
        oc = oc.reshape(ntile, NLAB, 2, L)
        sm = slot_maps[core]
        for t in range(ntile):
            for sl in range(2):
                ent = sm[2 * t + sl]
                if ent is None:
                    continue
                bb, r = ent
                raw[bb, :, L * r:L * (r + 1)] = oc[t, :, sl, :]

    # host-side tail: z768/769 rows of h plus b2, exact in fp32
    for b in range(B):
        s, e = spans[b]
        iig = np.arange(L)[:, None]
        jjg = np.arange(L)[None, :]
        indb = np.where((iig >= s) & (iig <= jjg) & (jjg <= e), 1.0, 0.0)
        indb = indb + np.where((iig == s) & (jjg == e), 1.0, 0.0)
        zt = (Aj[b, None, :, 768:770] + Ai[b, :, None, 768:770]
              + b1[None, None, 768:770]
              + w1c[768:770][None, None, :] * indb[:, :, None])
        htail = np.maximum(zt, 0.0)                    # [L, L, 2]
        tail = htail @ W2T[768:770] + b2[None, None, :]  # [L, L, 36]
        raw[b] += tail.reshape(L * L, NLAB).T

    mask_flat = maskf.reshape(-1)[None, None, :]            # [1,1,L*L]
    n_invalid = float(L * L - maskf.sum())
    ex = np.exp(raw.astype(np.float64)) * mask_flat
    total = ex.sum(axis=2) + n_invalid                      # [B, NLAB]
    lse = np.log(total).astype(np.float32)
    out_full = (raw * mask_flat - lse[:, :, None]).astype(np.float32)
    return out_full


# revision 37
# speedup vs baseline: 1.1111x; 1.1111x over previous
"""Trainium2 Bass kernel for the BERT span-pair classifier problem.

Computes, for B=2 batches over a 252x252 span-pair grid:
    h    = relu(Ai[i] + Aj[j] + ind(i,j)*w1c + b1)        # [770] per pair
    out  = h @ W2.T + b2                                   # [36]  per pair
    out  = where(span_mask >= 1, out, 0)
    res  = log_softmax(out over the 63504 pairs)           # per (batch, label)
    return res transposed to [B, 36, L*L]

v2 strategy (8 NeuronCores, SPMD single program):
  - Host precomputes Ai/Aj (O(L*770) matmuls) and ships per-core tensors:
      bj0b  : Aj + b1 per (chunk, batch, j)                       (shared)
      bjwt  : per-core in-span window rows with ai, w1c*ind baked  (static
              placement -> window overwrites are plain static relu-copies)
      aib   : per-(chunk, slot) Ai columns (tensor_scalar biases)
      hts6q : tail rows [h768, h769, m, 1-m] fully host-computed
      maskrep: mask broadcast over 36 partitions for the masked copy
      cnts  : per-core invalid-pair counts (for the softmax denominator)
  - Device work per 2-slot tile: 12 relu tensor_scalar ops (h build, spread
    over DVE/ACT/Pool via a greedy static load balancer), 6+1 bf16 matmuls
    (W2 chunks + host-baked tail with b2*m + BIGNEG*(1-m) rows), one packed
    exp+accum and one masked psum->SBUF multiply per 2-tile psum bank.
  - Two tiles share one PSUM bank (partitions 0-35 / 64-99), so exp /
    mask-mult / final(-LSE) ops run at [100, 504] granularity.
  - Each core ships raw per-group exp sums; the host combines them with the
    (host-known) invalid-pair counts, takes the log, and applies the final
    -LSE subtraction during unshard. No device collective, no device tail:
    stores stream out group by group during the main loop.
"""

import math
import os
from contextlib import ExitStack

import numpy as np

import concourse.bass as bass
import concourse.bacc as bacc
import concourse.tile as tile
from concourse import mybir
from concourse._compat import with_exitstack
from concourse.bass_utils import run_bass_kernel_spmd

L = 252
HID = 768
MLP = 770
NLAB = 36
B = 2
NC = 8
KC = 6            # full 128-row hid chunks (6*128 = 768)
W2SCALE = 16.0    # fp8 W2 is shipped pre-scaled; host divides logits by 16

FP32 = mybir.dt.float32
BF16 = mybir.dt.bfloat16
FP8 = mybir.dt.float8e4
AF = mybir.ActivationFunctionType
ALU = mybir.AluOpType

SLOTW = 252       # per-slot j width inside an h tile


def plan_slots(spans):
    """Slot layout: [in0, off0(+pad), in1, off1(+pad)]; per-batch slot count
    padded to a multiple of 4 so 2-tile psum groups are batch-pure."""
    segs = []
    slot = 0
    for b in range(B):
        s, e = spans[b]
        n = e - s + 1
        nin = math.ceil(n / NC)
        noff = math.ceil((L - n) / NC)
        pad = (-(nin + noff)) % 4
        segs.append(dict(kind="in", b=b, start=slot, nslots=nin, s=s, e=e,
                         count=n))
        slot += nin
        rows = [r for r in range(L) if r < s or r > e]
        segs.append(dict(kind="off", b=b, start=slot, nslots=noff + pad,
                         rows=rows, count=len(rows)))
        slot += noff + pad
    nslot = slot
    assert nslot % 4 == 0
    return segs, nslot


def slot_map_for_core(segs, nslot, c):
    """-> list over slots of (batch, global_row) or None for padding."""
    m = [None] * nslot
    for sg in segs:
        for k in range(sg["nslots"]):
            idx = NC * k + c
            p = sg["start"] + k
            if idx < sg["count"]:
                if sg["kind"] == "in":
                    m[p] = (sg["b"], sg["s"] + idx)
                else:
                    m[p] = (sg["b"], sg["rows"][idx])
    return m


def window_layout(segs):
    """Static (compile-time) ragged layout of the in-span window rows.

    Returns list of (slot, batch, k, j0, W, woff) and total width WTOT.
    Window for in-span slot k of batch b: columns [j0, j0+W) with
    j0 = s + 8k, W = min(e - s - 8k + 9, SLOTW - j0), covering [i_c, e]
    for every core offset c in [0, 8).
    """
    ents = []
    off = 0
    for sg in segs:
        if sg["kind"] != "in":
            continue
        s, e = sg["s"], sg["e"]
        for k in range(sg["nslots"]):
            j0 = s + NC * k
            W = min(e - s - NC * k + 9, SLOTW - j0)
            W = max(W, 1)
            ents.append(dict(slot=sg["start"] + k, b=sg["b"], k=k, j0=j0,
                             W=W, woff=off, s=s, e=e))
            off += W
    return ents, off


def build_kernel(segs, nslot, wents, WTOT, plan):
    ntile = nslot // 2
    ngrp = ntile // 2
    slot_batch = []
    for sg in segs:
        slot_batch.extend([sg["b"]] * sg["nslots"])
    grp_batch = [slot_batch[4 * g] for g in range(ngrp)]
    for g in range(ngrp):
        assert len({slot_batch[4 * g + i] for i in range(4)}) == 1
    # batch -> contiguous group range
    b0g = sum(1 for b in grp_batch if b == 0)
    assert all(b == 0 for b in grp_batch[:b0g])
    wents_by_slot = {w["slot"]: w for w in wents}

    @with_exitstack
    def kern(ctx: ExitStack, tc: tile.TileContext, outs, ins):
        nc = tc.nc
        bj0b = ins["bj0b"]        # [6*128, 1024] bf16 (chunk-major)
        bjwt = ins["bjwt"]        # [6*128, WTOT] bf16 (chunk-major)
        aib = ins["aib"]          # [6*128, nslot] bf16 (chunk-major)
        w2c6 = ins["w2c6"]        # [128, 3*2*64] fp8 (DoubleRow pairs, x16)
        w2b = ins["w2b"]          # [128, 6*36] bf16 (x16)
        outd = outs["out"]        # [ntile*36, 504] bf16 (tile-major rows)

        fp = ctx.enter_context(tc.tile_pool(name="fp", bufs=1))
        hp = ctx.enter_context(tc.tile_pool(name="hp", bufs=4))
        psp = ctx.enter_context(tc.tile_pool(name="psp", bufs=1, space="PSUM"))

        # ---- persistent SBUF ----
        s_bj0 = fp.tile([128, KC * 1024], BF16)
        s_bjwb = fp.tile([128, KC * WTOT], BF16)
        s_ai = fp.tile([128, KC * nslot], FP32)
        s_aib = fp.tile([128, KC * nslot], BF16)
        s_w2 = fp.tile([128, KC * 64], FP8)
        s_w2b = fp.tile([128, KC * NLAB], BF16)
        s_out = fp.tile([NLAB, ntile * 504], BF16)

        # ---- load constants (many small pieces -> spread over DMA queues,
        # ordered so early-group data lands first) ----
        q = [nc.sync, nc.gpsimd]
        nq = 0

        def ld(dst, src):
            nonlocal nq
            q[nq % 2].dma_start(out=dst, in_=src)
            nq += 1

        ld(s_w2, w2c6)
        ld(s_w2b, w2b)
        WP = 3

        def ldw(c, wpc):
            w0 = (WTOT // WP) * wpc
            w1 = (WTOT // WP) * (wpc + 1) if wpc < WP - 1 else WTOT
            ld(s_bjwb[:, WTOT * c + w0:WTOT * c + w1],
               bjwt[128 * c:128 * (c + 1), w0:w1])

        for c in range(KC):
            ld(s_aib[:, nslot * c:nslot * (c + 1)],
               aib[128 * c:128 * (c + 1), :])
            nc.vector.tensor_copy(out=s_ai[:, nslot * c:nslot * (c + 1)],
                                  in_=s_aib[:, nslot * c:nslot * (c + 1)])
            ld(s_bj0[:, 1024 * c:1024 * (c + 1)],
               bj0b[128 * c:128 * (c + 1), :])
            ldw(c, 0)
        for m in range(WP - 1):
            for c in range(KC):
                ldw(c, m + 1)

        def ts_relu(eng, out, in0, sc):
            if eng is nc.scalar:
                nc.scalar.activation(out, in0, AF.Relu, bias=sc, scale=1.0)
            else:
                eng.tensor_scalar(out=out, in0=in0, scalar1=sc, scalar2=0.0,
                                  op0=ALU.add, op1=ALU.max)

        # ---- main loop over tiles: one psum bank per tile ----
        psb = [psp.tile([NLAB, 2 * L], FP32, tag=f"ps{i}", name=f"ps{i}")
               for i in range(8)]

        def emit_copy(tc_):
            psc = psb[tc_ % 8]
            ceng = plan(tc_, 0, 0)
            seg = s_out[:, 2 * L * tc_:2 * L * (tc_ + 1)]
            if ceng is nc.scalar:
                nc.scalar.activation(seg, psc, AF.Identity, scale=1.0)
            else:
                nc.vector.tensor_copy(out=seg, in_=psc)
            t1 = tc_ + 1
            if t1 % 4 == 0 or t1 == ntile:
                t0 = t1 - (4 if t1 % 4 == 0 else t1 % 4)
                nt = t1 - t0
                dst = bass.AP(tensor=outd.tensor,
                              offset=outd.offset + NLAB * t0 * 504,
                              ap=[[504, NLAB], [NLAB * 504, nt], [1, 504]])
                nc.gpsimd.dma_start(out=dst,
                                    in_=s_out[:, 2 * L * t0:2 * L * t1])

        for t in range(ntile):
            ps = psb[t % 8]
            fp8tile = plan(t, 0, 0) is nc.scalar
            if fp8tile:
                hh = hp.tile([128, KC * 512], FP8, tag="hh8",
                             name=f"hh{t}")
            else:
                hh = hp.tile([128, KC * 512], BF16, tag="hhb",
                             name=f"hh{t}")
            for sl in range(2):
                p = 2 * t + sl
                b = slot_batch[p]
                went = wents_by_slot.get(p)
                for c in range(KC):
                    eng = plan(t, c, sl)
                    ho = 512 * c + SLOTW * sl
                    ts_relu(eng, hh[:, ho:ho + L],
                            s_bj0[:, 1024 * c + 512 * b:
                                  1024 * c + 512 * b + L],
                            s_ai[:, nslot * c + p:nslot * c + p + 1])
                    if went is not None:
                        # window rows are shipped pre-relu'd fp8: the
                        # overwrite is a plain copy (max(x,0) is a no-op)
                        wo = WTOT * c + went["woff"]
                        weng = plan(t, c, sl + 2)
                        wdst = hh[:, ho + went["j0"]:
                                  ho + went["j0"] + went["W"]]
                        wsrc = s_bjwb[:, wo:wo + went["W"]]
                        if weng is nc.scalar:
                            nc.scalar.activation(wdst, wsrc, AF.Relu,
                                                 scale=1.0)
                        else:
                            weng.tensor_scalar(
                                out=wdst, in0=wsrc, scalar1=0.0,
                                scalar2=None, op0=ALU.max)
            if fp8tile:
                # 3 fp8 DoubleRow chunk-pair matmuls
                for qq in range(KC // 2):
                    rhs = hh[:, 1024 * qq:1024 * (qq + 1)].rearrange(
                        "p (k sw) -> p k sw", k=2)[:, :, 0:2 * L]
                    lhs = s_w2[:, 128 * qq:128 * (qq + 1)].rearrange(
                        "p (k n) -> p k n", k=2)[:, :, 0:NLAB]
                    nc.tensor.matmul(
                        ps, lhs, rhs,
                        perf_mode=mybir.MatmulPerfMode.DoubleRow,
                        start=(qq == 0), stop=(qq == KC // 2 - 1),
                        skip_group_check=True)
            else:
                # 6 bf16 chunk matmuls
                for c in range(KC):
                    rhs = hh[:, 512 * c:512 * c + 2 * L]
                    nc.tensor.matmul(ps, s_w2b[:, NLAB * c:NLAB * (c + 1)],
                                     rhs,
                                     start=(c == 0), stop=(c == KC - 1),
                                     skip_group_check=True)
            # raw partial logits -> bf16 staging with a 2-tile emission
            # delay (the psum read then never waits at an engine queue
            # head), then straight out to HBM; the 770-tail + b2, mask,
            # exp-sums and -LSE are applied host-side
            if t >= 2:
                emit_copy(t - 2)
        emit_copy(ntile - 2)
        emit_copy(ntile - 1)

    return kern, ngrp


def make_plan(nc_getter, segs, nslot, wents):
    """Greedy static load balancer for the h-build ops."""
    ntile = nslot // 2
    wents_by_slot = {w["slot"]: w for w in wents}
    # preload other duties (ns): ACT: exp+copy+accum; DVE: memsets+casts
    # Pool excluded: measured ~4.6us per tensor op (Q7 emulation).
    # Tile-granular assignment: all h ops of a tile go to ONE engine so the
    # consuming matmuls' waits are satisfied in issue order (no sequencer
    # head-of-line blocking on scattered cross-engine deps).
    load = {"v": 2000.0, "a": 1500.0}

    table = {}
    for t in range(ntile):
        wins = [wents_by_slot[2 * t + sl] for sl in range(2)
                if (2 * t + sl) in wents_by_slot]
        cv = 12 * 204.0 + 548.0 + sum(
            KC * (0.52 * w["W"] + 60.0) for w in wins)
        ca = 12 * 350.0 + 674.0 + sum(
            KC * (0.833 * w["W"] + 110.0) for w in wins)
        k = "v" if load["v"] + cv <= load["a"] + ca else "a"
        load[k] += cv if k == "v" else ca
        table[t] = k

    def plan(t, c, sl):
        nc = nc_getter()
        return {"v": nc.vector, "a": nc.scalar}[table[t]]

    return plan


def kernel(**inputs) -> np.ndarray:
    hidden = np.asarray(inputs["hidden"], dtype=np.float32)
    pred_spans = np.asarray(inputs["pred_spans"]).astype(np.int64)
    span_mask = np.asarray(inputs["span_mask"]).astype(np.int32)
    W1 = np.asarray(inputs["W1"], dtype=np.float32)
    b1 = np.asarray(inputs["b1"], dtype=np.float32)
    W2 = np.asarray(inputs["W2"], dtype=np.float32)
    b2 = np.asarray(inputs["b2"], dtype=np.float32)

    spans = [(int(pred_spans[b, 0]), int(pred_spans[b, 1])) for b in range(B)]
    segs, nslot = plan_slots(spans)
    ntile = nslot // 2
    ngrp = ntile // 2
    wents, WTOT = window_layout(segs)
    wents_by_slot = {w["slot"]: w for w in wents}

    vecs = hidden[:, 1:L + 1, :]                       # [B, L, 768]
    W1T = W1.T                                         # [1537, 770]
    w1c = np.ascontiguousarray(W1T[2 * HID])           # [770]
    # host prep: Ai/Aj for all rows/cols
    Aj = np.einsum("bld,dh->blh", vecs, W1T[HID:2 * HID])   # [B, L, 770]
    Ai = np.einsum("bld,dh->blh", vecs, W1T[0:HID])         # [B, L, 770]
    Bj0 = Aj + b1[None, None, :]                            # [B, L, 770]

    W2T = np.ascontiguousarray(W2.T)                   # [770, 36]
    maskf = span_mask.astype(np.float32).clip(0, 1)    # [252, 252]

    bf = mybir.dt.np(BF16)
    f8 = mybir.dt.np(FP8)

    # shared tensors
    bj0b = np.zeros((128, KC, 2, 512), np.float32)
    for c in range(KC):
        for b in range(B):
            bj0b[:, c, b, 0:L] = Bj0[b, :, 128 * c:128 * (c + 1)].T
    bj0b = np.ascontiguousarray(
        bj0b.transpose(1, 0, 2, 3)).reshape(KC * 128, 1024)

    # fp8 DoubleRow stationary: (p, pair q, k, n) = W2T[256q + 128k + p, n],
    # k-stride padded to 64 cols for the 16B ldweights alignment rule
    w2c6 = np.zeros((128, KC // 2, 2, 64), np.float32)
    for qq in range(KC // 2):
        for k in range(2):
            r0 = 256 * qq + 128 * k
            w2c6[:, qq, k, 0:NLAB] = W2T[r0:r0 + 128] * W2SCALE
    w2c6 = w2c6.reshape(128, KC * 64)
    w2b = np.zeros((128, KC, NLAB), np.float32)
    for c in range(KC):
        w2b[:, c, :] = W2T[128 * c:128 * (c + 1)] * W2SCALE
    w2b = w2b.reshape(128, KC * NLAB)

    in_maps = []
    slot_maps = []
    core_cnts = []
    for core in range(NC):
        sm = slot_map_for_core(segs, nslot, core)
        slot_maps.append(sm)

        # aib: per-(chunk, slot) Ai columns
        aib = np.zeros((128, KC, nslot), np.float32)
        for p, ent in enumerate(sm):
            if ent is None:
                continue
            b, r = ent
            for c in range(KC):
                aib[:, c, p] = Ai[b, r, 128 * c:128 * (c + 1)]

        # bjwt: in-span window rows, everything baked (ai + w1c*ind [+E2])
        bjwt = np.zeros((128, KC, WTOT), np.float32)
        for w in wents:
            b = w["b"]
            s, e, k = w["s"], w["e"], w["k"]
            i = s + NC * k + core
            ent = sm[w["slot"]]
            js = np.arange(w["j0"], w["j0"] + w["W"])
            jc = np.clip(js, 0, L - 1)
            ind = ((js >= i) & (js <= e)).astype(np.float32)
            if k == 0 and core == 0:
                ind[js == e] = 2.0
            valid = (js < L).astype(np.float32)
            if ent is None:
                ai_row = np.zeros((MLP,), np.float32)
                ind = ind * 0.0
            else:
                ai_row = Ai[b, i]
            for c in range(KC):
                rows = slice(128 * c, 128 * (c + 1))
                vals = (Bj0[b, jc, 128 * c:128 * (c + 1)].T
                        + ai_row[rows, None]
                        + w1c[rows, None] * ind[None, :]) * valid[None, :]
                bjwt[:, c, w["woff"]:w["woff"] + w["W"]] = np.maximum(vals, 0)
        bjwt = np.ascontiguousarray(bjwt.transpose(1, 0, 2)).reshape(KC * 128, WTOT)

        # invalid-pair counts for this core's real rows, per batch
        cnt = np.zeros((2,), np.float64)
        for p, ent in enumerate(sm):
            if ent is None:
                continue
            b, r = ent
            cnt[b] += L - maskf[r].sum()

        core_cnts.append(cnt)
        in_maps.append({
            "bj0b": bj0b.astype(bf), "bjwt": bjwt.astype(bf),
            "aib": np.ascontiguousarray(
                aib.transpose(1, 0, 2)).reshape(KC * 128, nslot).astype(bf),
            "w2c6": w2c6.astype(f8), "w2b": w2b.astype(bf),
        })

    # ---- build program ----
    nc = bacc.Bacc("TRN2", target_bir_lowering=False, debug=False,
                   enable_asserts=False, num_devices=NC)

    def mk(name, arr, dt):
        return nc.dram_tensor(name, list(arr.shape), dt,
                              kind="ExternalInput").ap()

    ex = in_maps[0]
    ins_aps = {
        "bj0b": mk("bj0b", ex["bj0b"], BF16),
        "bjwt": mk("bjwt", ex["bjwt"], BF16),
        "aib": mk("aib", ex["aib"], BF16),
        "w2c6": mk("w2c6", ex["w2c6"], FP8),
        "w2b": mk("w2b", ex["w2b"], BF16),
    }
    outs_aps = {
        "out": nc.dram_tensor("out", [(nslot // 2) * NLAB, 504], BF16,
                              kind="ExternalOutput").ap(),
    }

    plan = make_plan(lambda: nc, segs, nslot, wents)
    kern, ngrp_chk = build_kernel(segs, nslot, wents, WTOT, plan)
    with tile.TileContext(nc) as t:
        kern(t, outs_aps, ins_aps)
    nc.compile()

    if os.environ.get("BK_BUILD_ONLY"):
        print("BUILD OK")
        return np.zeros((B, NLAB, L * L), np.float32)

    if os.environ.get("BK_SIM"):
        from concourse.bass_interp import MultiCoreSim

        sim = MultiCoreSim(nc, num_cores=NC, require_finite=False,
                           require_nnan=False)
        for c, cs in sim.cores.items():
            for name, arr in in_maps[c].items():
                cs.tensor(name)[:] = arr
            if nc.partition_id_tensor is not None:
                cs.tensor(nc.partition_id_tensor.name)[:] = np.array(
                    [[c]], dtype=np.uint32)
        sim.simulate(check_with_hw=False)

        class _R:
            results = [{"out": np.asarray(sim.cores[c].tensor("out"))}
                       for c in range(NC)]
        res = _R()
    else:
        trace = bool(int(os.environ.get("BK_TRACE", "0")))
        res = run_bass_kernel_spmd(nc, in_maps, core_ids=list(range(NC)),
                                   trace=trace)
        if trace and res.exec_time_ns is not None:
            print(f"HW exec time: {res.exec_time_ns} ns")

    # ---- unshard + host-side mask / log-sum-exp / -LSE ----
    ngrp = nslot // 4
    slot_batch = []
    for sg in segs:
        slot_batch.extend([sg["b"]] * sg["nslots"])
    grp_batch = [slot_batch[4 * g] for g in range(ngrp)]

    raw = np.zeros((B, NLAB, L * L), np.float32)
    ntile = nslot // 2
    for core in range(NC):
        oc = res.results[core]["out"].astype(np.float32) / W2SCALE
        oc = oc.reshape(ntile, NLAB, 2, L)
        sm = slot_maps[core]
        for t in range(ntile):
            for sl in range(2):
                ent = sm[2 * t + sl]
                if ent is None:
                    continue
                bb, r = ent
                raw[bb, :, L * r:L * (r + 1)] = oc[t, :, sl, :]

    # host-side tail: z768/769 rows of h plus b2, exact in fp32
    for b in range(B):
        s, e = spans[b]
        iig = np.arange(L)[:, None]
        jjg = np.arange(L)[None, :]
        indb = np.where((iig >= s) & (iig <= jjg) & (jjg <= e), 1.0, 0.0)
        indb = indb + np.where((iig == s) & (jjg == e), 1.0, 0.0)
        zt = (Aj[b, None, :, 768:770] + Ai[b, :, None, 768:770]
              + b1[None, None, 768:770]
              + w1c[768:770][None, None, :] * indb[:, :, None])
        htail = np.maximum(zt, 0.0)                    # [L, L, 2]
        tail = htail @ W2T[768:770] + b2[None, None, :]  # [L, L, 36]
        raw[b] += tail.reshape(L * L, NLAB).T

    mask_flat = maskf.reshape(-1)[None, None, :]            # [1,1,L*L]
    n_invalid = float(L * L - maskf.sum())
    ex = np.exp(raw.astype(np.float64)) * mask_flat
    total = ex.sum(axis=2) + n_invalid                      # [B, NLAB]
    lse = np.log(total).astype(np.float32)
    out_full = (raw * mask_flat - lse[:, :, None]).astype(np.float32)
    return out_full


# revision 38
# speedup vs baseline: 1.2424x; 1.1182x over previous
"""Trainium2 Bass kernel for the BERT span-pair classifier problem.

Computes, for B=2 batches over a 252x252 span-pair grid:
    h    = relu(Ai[i] + Aj[j] + ind(i,j)*w1c + b1)        # [770] per pair
    out  = h @ W2.T + b2                                   # [36]  per pair
    out  = where(span_mask >= 1, out, 0)
    res  = log_softmax(out over the 63504 pairs)           # per (batch, label)
    return res transposed to [B, 36, L*L]

v2 strategy (8 NeuronCores, SPMD single program):
  - Host precomputes Ai/Aj (O(L*770) matmuls) and ships per-core tensors:
      bj0b  : Aj + b1 per (chunk, batch, j)                       (shared)
      bjwt  : per-core in-span window rows with ai, w1c*ind baked  (static
              placement -> window overwrites are plain static relu-copies)
      aib   : per-(chunk, slot) Ai columns (tensor_scalar biases)
      hts6q : tail rows [h768, h769, m, 1-m] fully host-computed
      maskrep: mask broadcast over 36 partitions for the masked copy
      cnts  : per-core invalid-pair counts (for the softmax denominator)
  - Device work per 2-slot tile: 12 relu tensor_scalar ops (h build, spread
    over DVE/ACT/Pool via a greedy static load balancer), 6+1 bf16 matmuls
    (W2 chunks + host-baked tail with b2*m + BIGNEG*(1-m) rows), one packed
    exp+accum and one masked psum->SBUF multiply per 2-tile psum bank.
  - Two tiles share one PSUM bank (partitions 0-35 / 64-99), so exp /
    mask-mult / final(-LSE) ops run at [100, 504] granularity.
  - Each core ships raw per-group exp sums; the host combines them with the
    (host-known) invalid-pair counts, takes the log, and applies the final
    -LSE subtraction during unshard. No device collective, no device tail:
    stores stream out group by group during the main loop.
"""

import math
import os
from contextlib import ExitStack

import numpy as np

import concourse.bass as bass
import concourse.bacc as bacc
import concourse.tile as tile
from concourse import mybir
from concourse._compat import with_exitstack
from concourse.bass_utils import run_bass_kernel_spmd

L = 252
HID = 768
MLP = 770
NLAB = 36
B = 2
NC = 8
KC = 6            # full 128-row hid chunks (6*128 = 768)
W2SCALE = 16.0    # fp8 W2 is shipped pre-scaled; host divides logits by 16

FP32 = mybir.dt.float32
BF16 = mybir.dt.bfloat16
FP8 = mybir.dt.float8e4
AF = mybir.ActivationFunctionType
ALU = mybir.AluOpType

SLOTW = 252       # per-slot j width inside an h tile


def plan_slots(spans):
    """Slot layout: [in0, off0(+pad), in1, off1(+pad)]; per-batch slot count
    padded to a multiple of 4 so 2-tile psum groups are batch-pure."""
    segs = []
    slot = 0
    for b in range(B):
        s, e = spans[b]
        n = e - s + 1
        nin = math.ceil(n / NC)
        noff = math.ceil((L - n) / NC)
        pad = (-(nin + noff)) % 4
        segs.append(dict(kind="in", b=b, start=slot, nslots=nin, s=s, e=e,
                         count=n))
        slot += nin
        rows = [r for r in range(L) if r < s or r > e]
        segs.append(dict(kind="off", b=b, start=slot, nslots=noff + pad,
                         rows=rows, count=len(rows)))
        slot += noff + pad
    nslot = slot
    assert nslot % 4 == 0
    return segs, nslot


def slot_map_for_core(segs, nslot, c):
    """-> list over slots of (batch, global_row) or None for padding."""
    m = [None] * nslot
    for sg in segs:
        for k in range(sg["nslots"]):
            idx = NC * k + c
            p = sg["start"] + k
            if idx < sg["count"]:
                if sg["kind"] == "in":
                    m[p] = (sg["b"], sg["s"] + idx)
                else:
                    m[p] = (sg["b"], sg["rows"][idx])
    return m


def window_layout(segs):
    """Static (compile-time) ragged layout of the in-span window rows.

    Returns list of (slot, batch, k, j0, W, woff) and total width WTOT.
    Window for in-span slot k of batch b: columns [j0, j0+W) with
    j0 = s + 8k, W = min(e - s - 8k + 9, SLOTW - j0), covering [i_c, e]
    for every core offset c in [0, 8).
    """
    ents = []
    off = 0
    for sg in segs:
        if sg["kind"] != "in":
            continue
        s, e = sg["s"], sg["e"]
        for k in range(sg["nslots"]):
            j0 = s + NC * k
            W = min(e - s - NC * k + 9, SLOTW - j0)
            W = max(W, 1)
            ents.append(dict(slot=sg["start"] + k, b=sg["b"], k=k, j0=j0,
                             W=W, woff=off, s=s, e=e))
            off += W
    return ents, off


def build_kernel(segs, nslot, wents, WTOT, plan):
    ntile = nslot // 2
    ngrp = ntile // 2
    slot_batch = []
    for sg in segs:
        slot_batch.extend([sg["b"]] * sg["nslots"])
    grp_batch = [slot_batch[4 * g] for g in range(ngrp)]
    for g in range(ngrp):
        assert len({slot_batch[4 * g + i] for i in range(4)}) == 1
    # batch -> contiguous group range
    b0g = sum(1 for b in grp_batch if b == 0)
    assert all(b == 0 for b in grp_batch[:b0g])
    wents_by_slot = {w["slot"]: w for w in wents}

    @with_exitstack
    def kern(ctx: ExitStack, tc: tile.TileContext, outs, ins):
        nc = tc.nc
        bj0b = ins["bj0b"]        # [6*128, 1024] bf16 (chunk-major)
        bjwt = ins["bjwt"]        # [6*128, WTOT] bf16 (chunk-major)
        aib = ins["aib"]          # [6*128, nslot] bf16 (chunk-major)
        w2c6 = ins["w2c6"]        # [128, 3*2*64] fp8 (DoubleRow pairs, x16)
        w2b = ins["w2b"]          # [128, 6*36] bf16 (x16)
        outd = outs["out"]        # [ntile*36, 504] bf16 (tile-major rows)

        fp = ctx.enter_context(tc.tile_pool(name="fp", bufs=1))
        hp = ctx.enter_context(tc.tile_pool(name="hp", bufs=4))
        psp = ctx.enter_context(tc.tile_pool(name="psp", bufs=1, space="PSUM"))

        # ---- persistent SBUF ----
        s_bj0 = fp.tile([128, KC * 1024], BF16)
        s_bjwb = fp.tile([128, KC * WTOT], BF16)
        s_ai = fp.tile([128, KC * nslot], FP32)
        s_aib = fp.tile([128, KC * nslot], BF16)
        s_w2 = fp.tile([128, KC * 64], FP8)
        s_w2b = fp.tile([128, KC * NLAB], BF16)
        s_out = fp.tile([NLAB, ntile * 504], BF16)

        # ---- load constants (many small pieces -> spread over DMA queues,
        # ordered so early-group data lands first) ----
        q = [nc.sync, nc.gpsimd]
        nq = 0

        def ld(dst, src):
            nonlocal nq
            q[nq % 2].dma_start(out=dst, in_=src)
            nq += 1

        ld(s_w2, w2c6)
        ld(s_w2b, w2b)
        WP = 3

        def ldw(c, wpc):
            w0 = (WTOT // WP) * wpc
            w1 = (WTOT // WP) * (wpc + 1) if wpc < WP - 1 else WTOT
            ld(s_bjwb[:, WTOT * c + w0:WTOT * c + w1],
               bjwt[128 * c:128 * (c + 1), w0:w1])

        for c in range(KC):
            ld(s_aib[:, nslot * c:nslot * (c + 1)],
               aib[128 * c:128 * (c + 1), :])
            nc.vector.tensor_copy(out=s_ai[:, nslot * c:nslot * (c + 1)],
                                  in_=s_aib[:, nslot * c:nslot * (c + 1)])
            ld(s_bj0[:, 1024 * c:1024 * (c + 1)],
               bj0b[128 * c:128 * (c + 1), :])
            ldw(c, 0)
        for m in range(WP - 1):
            for c in range(KC):
                ldw(c, m + 1)

        def ts_relu(eng, out, in0, sc):
            if eng is nc.scalar:
                nc.scalar.activation(out, in0, AF.Relu, bias=sc, scale=1.0)
            else:
                eng.tensor_scalar(out=out, in0=in0, scalar1=sc, scalar2=0.0,
                                  op0=ALU.add, op1=ALU.max)

        # ---- main loop over tiles: one psum bank per tile ----
        psb = [psp.tile([NLAB, 2 * L], FP32, tag=f"ps{i}", name=f"ps{i}")
               for i in range(8)]

        def emit_copy(tc_):
            psc = psb[tc_ % 8]
            ceng = plan(tc_, 0, 0)
            seg = s_out[:, 2 * L * tc_:2 * L * (tc_ + 1)]
            if ceng is nc.scalar:
                nc.scalar.activation(seg, psc, AF.Identity, scale=1.0)
            else:
                nc.vector.tensor_copy(out=seg, in_=psc)
            t1 = tc_ + 1
            if t1 % 4 == 0 or t1 == ntile:
                t0 = t1 - (4 if t1 % 4 == 0 else t1 % 4)
                nt = t1 - t0
                dst = bass.AP(tensor=outd.tensor,
                              offset=outd.offset + NLAB * t0 * 504,
                              ap=[[504, NLAB], [NLAB * 504, nt], [1, 504]])
                nc.gpsimd.dma_start(out=dst,
                                    in_=s_out[:, 2 * L * t0:2 * L * t1])

        for t in range(ntile):
            ps = psb[t % 8]
            fp8tile = plan(t, 0, 0) is nc.scalar
            if fp8tile:
                hh = hp.tile([128, KC * 512], FP8, tag="hh8",
                             name=f"hh{t}")
            else:
                hh = hp.tile([128, KC * 512], BF16, tag="hhb",
                             name=f"hh{t}")
            for sl in range(2):
                p = 2 * t + sl
                b = slot_batch[p]
                went = wents_by_slot.get(p)
                for c in range(KC):
                    eng = plan(t, c, sl)
                    ho = 512 * c + SLOTW * sl
                    ts_relu(eng, hh[:, ho:ho + L],
                            s_bj0[:, 1024 * c + 512 * b:
                                  1024 * c + 512 * b + L],
                            s_ai[:, nslot * c + p:nslot * c + p + 1])
                    if went is not None:
                        # window rows are shipped pre-relu'd fp8: the
                        # overwrite is a plain copy (max(x,0) is a no-op)
                        wo = WTOT * c + went["woff"]
                        weng = plan(t, c, sl + 2)
                        wdst = hh[:, ho + went["j0"]:
                                  ho + went["j0"] + went["W"]]
                        wsrc = s_bjwb[:, wo:wo + went["W"]]
                        if weng is nc.scalar:
                            nc.scalar.activation(wdst, wsrc, AF.Relu,
                                                 scale=1.0)
                        else:
                            weng.tensor_scalar(
                                out=wdst, in0=wsrc, scalar1=0.0,
                                scalar2=None, op0=ALU.max)
            if fp8tile:
                # 3 fp8 DoubleRow chunk-pair matmuls
                for qq in range(KC // 2):
                    rhs = hh[:, 1024 * qq:1024 * (qq + 1)].rearrange(
                        "p (k sw) -> p k sw", k=2)[:, :, 0:2 * L]
                    lhs = s_w2[:, 128 * qq:128 * (qq + 1)].rearrange(
                        "p (k n) -> p k n", k=2)[:, :, 0:NLAB]
                    nc.tensor.matmul(
                        ps, lhs, rhs,
                        perf_mode=mybir.MatmulPerfMode.DoubleRow,
                        start=(qq == 0), stop=(qq == KC // 2 - 1),
                        skip_group_check=True)
            else:
                # 6 bf16 chunk matmuls
                for c in range(KC):
                    rhs = hh[:, 512 * c:512 * c + 2 * L]
                    nc.tensor.matmul(ps, s_w2b[:, NLAB * c:NLAB * (c + 1)],
                                     rhs,
                                     start=(c == 0), stop=(c == KC - 1),
                                     skip_group_check=True)
            # raw partial logits -> bf16 staging with a 2-tile emission
            # delay (the psum read then never waits at an engine queue
            # head), then straight out to HBM; the 770-tail + b2, mask,
            # exp-sums and -LSE are applied host-side
            if t >= 2:
                emit_copy(t - 2)
        emit_copy(ntile - 2)
        emit_copy(ntile - 1)

    return kern, ngrp


def make_plan(nc_getter, segs, nslot, wents):
    """Greedy static load balancer for the h-build ops."""
    ntile = nslot // 2
    wents_by_slot = {w["slot"]: w for w in wents}
    # preload other duties (ns): ACT: exp+copy+accum; DVE: memsets+casts
    # Pool excluded: measured ~4.6us per tensor op (Q7 emulation).
    # Tile-granular assignment: all h ops of a tile go to ONE engine so the
    # consuming matmuls' waits are satisfied in issue order (no sequencer
    # head-of-line blocking on scattered cross-engine deps).
    load = {"v": 2500.0, "a": 10000.0}

    table = {}
    for t in range(ntile):
        wins = [wents_by_slot[2 * t + sl] for sl in range(2)
                if (2 * t + sl) in wents_by_slot]
        cv = 12 * 194.0 + 499.0 + sum(
            KC * (0.52 * w["W"] + 60.0) for w in wins)
        ca = 12 * 348.0 + 634.0 + sum(
            KC * (0.833 * w["W"] + 110.0) for w in wins)
        # keep the closing tiles on the fast engine so the tail drains fast
        if t >= ntile - 4:
            k = "v"
        else:
            k = "v" if load["v"] + cv <= load["a"] + ca else "a"
        load[k] += cv if k == "v" else ca
        table[t] = k

    def plan(t, c, sl):
        nc = nc_getter()
        return {"v": nc.vector, "a": nc.scalar}[table[t]]

    return plan


def kernel(**inputs) -> np.ndarray:
    hidden = np.asarray(inputs["hidden"], dtype=np.float32)
    pred_spans = np.asarray(inputs["pred_spans"]).astype(np.int64)
    span_mask = np.asarray(inputs["span_mask"]).astype(np.int32)
    W1 = np.asarray(inputs["W1"], dtype=np.float32)
    b1 = np.asarray(inputs["b1"], dtype=np.float32)
    W2 = np.asarray(inputs["W2"], dtype=np.float32)
    b2 = np.asarray(inputs["b2"], dtype=np.float32)

    spans = [(int(pred_spans[b, 0]), int(pred_spans[b, 1])) for b in range(B)]
    segs, nslot = plan_slots(spans)
    ntile = nslot // 2
    ngrp = ntile // 2
    wents, WTOT = window_layout(segs)
    wents_by_slot = {w["slot"]: w for w in wents}

    vecs = hidden[:, 1:L + 1, :]                       # [B, L, 768]
    W1T = W1.T                                         # [1537, 770]
    w1c = np.ascontiguousarray(W1T[2 * HID])           # [770]
    # host prep: Ai/Aj for all rows/cols
    Aj = np.einsum("bld,dh->blh", vecs, W1T[HID:2 * HID])   # [B, L, 770]
    Ai = np.einsum("bld,dh->blh", vecs, W1T[0:HID])         # [B, L, 770]
    Bj0 = Aj + b1[None, None, :]                            # [B, L, 770]

    W2T = np.ascontiguousarray(W2.T)                   # [770, 36]
    maskf = span_mask.astype(np.float32).clip(0, 1)    # [252, 252]

    bf = mybir.dt.np(BF16)
    f8 = mybir.dt.np(FP8)

    # shared tensors
    bj0b = np.zeros((128, KC, 2, 512), np.float32)
    for c in range(KC):
        for b in range(B):
            bj0b[:, c, b, 0:L] = Bj0[b, :, 128 * c:128 * (c + 1)].T
    bj0b = np.ascontiguousarray(
        bj0b.transpose(1, 0, 2, 3)).reshape(KC * 128, 1024)

    # fp8 DoubleRow stationary: (p, pair q, k, n) = W2T[256q + 128k + p, n],
    # k-stride padded to 64 cols for the 16B ldweights alignment rule
    w2c6 = np.zeros((128, KC // 2, 2, 64), np.float32)
    for qq in range(KC // 2):
        for k in range(2):
            r0 = 256 * qq + 128 * k
            w2c6[:, qq, k, 0:NLAB] = W2T[r0:r0 + 128] * W2SCALE
    w2c6 = w2c6.reshape(128, KC * 64)
    w2b = np.zeros((128, KC, NLAB), np.float32)
    for c in range(KC):
        w2b[:, c, :] = W2T[128 * c:128 * (c + 1)] * W2SCALE
    w2b = w2b.reshape(128, KC * NLAB)

    in_maps = []
    slot_maps = []
    core_cnts = []
    for core in range(NC):
        sm = slot_map_for_core(segs, nslot, core)
        slot_maps.append(sm)

        # aib: per-(chunk, slot) Ai columns
        aib = np.zeros((128, KC, nslot), np.float32)
        for p, ent in enumerate(sm):
            if ent is None:
                continue
            b, r = ent
            for c in range(KC):
                aib[:, c, p] = Ai[b, r, 128 * c:128 * (c + 1)]

        # bjwt: in-span window rows, everything baked (ai + w1c*ind [+E2])
        bjwt = np.zeros((128, KC, WTOT), np.float32)
        for w in wents:
            b = w["b"]
            s, e, k = w["s"], w["e"], w["k"]
            i = s + NC * k + core
            ent = sm[w["slot"]]
            js = np.arange(w["j0"], w["j0"] + w["W"])
            jc = np.clip(js, 0, L - 1)
            ind = ((js >= i) & (js <= e)).astype(np.float32)
            if k == 0 and core == 0:
                ind[js == e] = 2.0
            valid = (js < L).astype(np.float32)
            if ent is None:
                ai_row = np.zeros((MLP,), np.float32)
                ind = ind * 0.0
            else:
                ai_row = Ai[b, i]
            for c in range(KC):
                rows = slice(128 * c, 128 * (c + 1))
                vals = (Bj0[b, jc, 128 * c:128 * (c + 1)].T
                        + ai_row[rows, None]
                        + w1c[rows, None] * ind[None, :]) * valid[None, :]
                bjwt[:, c, w["woff"]:w["woff"] + w["W"]] = np.maximum(vals, 0)
        bjwt = np.ascontiguousarray(bjwt.transpose(1, 0, 2)).reshape(KC * 128, WTOT)

        # invalid-pair counts for this core's real rows, per batch
        cnt = np.zeros((2,), np.float64)
        for p, ent in enumerate(sm):
            if ent is None:
                continue
            b, r = ent
            cnt[b] += L - maskf[r].sum()

        core_cnts.append(cnt)
        in_maps.append({
            "bj0b": bj0b.astype(bf), "bjwt": bjwt.astype(bf),
            "aib": np.ascontiguousarray(
                aib.transpose(1, 0, 2)).reshape(KC * 128, nslot).astype(bf),
            "w2c6": w2c6.astype(f8), "w2b": w2b.astype(bf),
        })

    # ---- build program ----
    nc = bacc.Bacc("TRN2", target_bir_lowering=False, debug=False,
                   enable_asserts=False, num_devices=NC)

    def mk(name, arr, dt):
        return nc.dram_tensor(name, list(arr.shape), dt,
                              kind="ExternalInput").ap()

    ex = in_maps[0]
    ins_aps = {
        "bj0b": mk("bj0b", ex["bj0b"], BF16),
        "bjwt": mk("bjwt", ex["bjwt"], BF16),
        "aib": mk("aib", ex["aib"], BF16),
        "w2c6": mk("w2c6", ex["w2c6"], FP8),
        "w2b": mk("w2b", ex["w2b"], BF16),
    }
    outs_aps = {
        "out": nc.dram_tensor("out", [(nslot // 2) * NLAB, 504], BF16,
                              kind="ExternalOutput").ap(),
    }

    plan = make_plan(lambda: nc, segs, nslot, wents)
    kern, ngrp_chk = build_kernel(segs, nslot, wents, WTOT, plan)
    with tile.TileContext(nc) as t:
        kern(t, outs_aps, ins_aps)
    nc.compile()

    if os.environ.get("BK_BUILD_ONLY"):
        print("BUILD OK")
        return np.zeros((B, NLAB, L * L), np.float32)

    if os.environ.get("BK_SIM"):
        from concourse.bass_interp import MultiCoreSim

        sim = MultiCoreSim(nc, num_cores=NC, require_finite=False,
                           require_nnan=False)
        for c, cs in sim.cores.items():
            for name, arr in in_maps[c].items():
                cs.tensor(name)[:] = arr
            if nc.partition_id_tensor is not None:
                cs.tensor(nc.partition_id_tensor.name)[:] = np.array(
                    [[c]], dtype=np.uint32)
        sim.simulate(check_with_hw=False)

        class _R:
            results = [{"out": np.asarray(sim.cores[c].tensor("out"))}
                       for c in range(NC)]
        res = _R()
    else:
        trace = bool(int(os.environ.get("BK_TRACE", "0")))
        res = run_bass_kernel_spmd(nc, in_maps, core_ids=list(range(NC)),
                                   trace=trace)
        if trace and res.exec_time_ns is not None:
            print(f"HW exec time: {res.exec_time_ns} ns")

    # ---- unshard + host-side mask / log-sum-exp / -LSE ----
    ngrp = nslot // 4
    slot_batch = []
    for sg in segs:
        slot_batch.extend([sg["b"]] * sg["nslots"])
    grp_batch = [slot_batch[4 * g] for g in range(ngrp)]

    raw = np.zeros((B, NLAB, L * L), np.float32)
    ntile = nslot // 2
    for core in range(NC):
        oc = res.results[core]["out"].astype(np.float32) / W2SCALE
        oc = oc.reshape(ntile, NLAB, 2, L)
        sm = slot_maps[core]
        for t in range(ntile):
            for sl in range(2):
                ent = sm[2 * t + sl]
                if ent is None:
                    continue
                bb, r = ent
                raw[bb, :, L * r:L * (r + 1)] = oc[t, :, sl, :]

    # host-side tail: z768/769 rows of h plus b2, exact in fp32
    for b in range(B):
        s, e = spans[b]
        iig = np.arange(L)[:, None]
        jjg = np.arange(L)[None, :]
        indb = np.where((iig >= s) & (iig <= jjg) & (jjg <= e), 1.0, 0.0)
        indb = indb + np.where((iig == s) & (jjg == e), 1.0, 0.0)
        zt = (Aj[b, None, :, 768:770] + Ai[b, :, None, 768:770]
              + b1[None, None, 768:770]
              + w1c[768:770][None, None, :] * indb[:, :, None])
        htail = np.maximum(zt, 0.0)                    # [L, L, 2]
        tail = htail @ W2T[768:770] + b2[None, None, :]  # [L, L, 36]
        raw[b] += tail.reshape(L * L, NLAB).T

    mask_flat = maskf.reshape(-1)[None, None, :]            # [1,1,L*L]
    n_invalid = float(L * L - maskf.sum())
    ex = np.exp(raw.astype(np.float64)) * mask_flat
    total = ex.sum(axis=2) + n_invalid                      # [B, NLAB]
    lse = np.log(total).astype(np.float32)
    out_full = (raw * mask_flat - lse[:, :, None]).astype(np.float32)
    return out_full
